# revision 13
# baseline (speedup 1.0000x reference)
"""Fused multi-head-free attention kernel for Trainium2 (Bass/Tile), 8-core SPMD.

Problem: nn_Attention — per batch element b:
    q = query[b] @ Wq + bq          [Sq, H]
    k = key[b]   @ Wk + bk          [Skv, H]
    v = value[b] @ Wv + bv          [Skv, H]
    S = q @ k.T                     [Sq, Skv]
    W = softmax(S, axis=-1)
    C = W @ v                       [Sq, H]
    returns (C, W)

Sharding: pure data-parallel over batch (B=8 == 8 cores), one batch element
per NeuronCore; projection weights replicated. No collectives.

Per-core dataflow (all matmuls on PE, fp32r for the numerically sensitive
path, bf16 for the post-softmax context path):
  1. x in {key, query, value} is DMA'd in 128-row tiles and transposed on PE
     (fp32r transpose via identity) into xT [D, S] layout in SBUF.
  2. qT/kT [H, S] = Wt.T @ xT (fp32r), bias added per-partition during the
     PSUM->SBUF copy.  v is computed in natural [Skv, H] layout (bf16 out).
  3. Per 128-row q-tile: S tile [128, Skv] = qT.T @ kT into 4 PSUM banks,
     row-max on DVE, exp(S - max) on ACT (with per-bank row-sum accumulation),
     normalize -> W (fp32, DMA'd out), and a bf16 copy of W is PE-transposed
     to feed C = W @ v accumulation (bf16), C copied out in fp32.
"""

import numpy as np

B, SQ, SKV, D, H = 8, 2048, 2048, 512, 512
P = 128                 # partitions
ST = SQ // P            # 16 s-tiles
DC = D // P             # 4 contraction chunks
HC = H // P             # 4 h tiles
NB = SKV // 512         # 4 psum banks per score row

_CACHE = {}


def _patch_multiwaits(nc, mb):
    """walrus in this container rejects >1 sync-wait per instruction
    (setupSyncWait: "Too many sync wait commands").  Split extra waits onto
    preceding same-engine NoOps — engine streams are in-order so semantics
    are preserved."""
    for blk in nc.m.functions[0].blocks:
        insts = list(blk.instructions)
        new_insts, changed = [], False
        for inst in insts:
            si = getattr(inst, "sync_info", None)
            if si is not None and si.on_wait and len(si.on_wait) > 1:
                waits = list(si.on_wait)
                extra, keep = waits[:-1], waits[-1:]
                for k, w in enumerate(extra):
                    new_insts.append(mb.InstNoOp(
                        name=f"{inst.name}-ws{k}",
                        sync_info=mb.SyncInfo(on_wait=[w], on_update=[]),
                        bass_nofuse=True, engine=inst.engine))
                si.on_wait = keep
                changed = True
            new_insts.append(inst)
        if changed:
            blk.instructions = new_insts


def _enable_ldw_opt():
    """Flip walrus --enable-ldw-opt to true (elides redundant LDWEIGHTS)."""
    from concourse import bass_utils as bu
    if getattr(bu, "_ldw_patched", False):
        return
    orig = bu.run_command

    def patched(cmd, **kw):
        cmd = [c.replace("--enable-ldw-opt=false", "--enable-ldw-opt=true")
               if isinstance(c, str) else c for c in cmd]
        return orig(cmd, **kw)

    bu.run_command = patched
    bu._ldw_patched = True


def _build():
    import concourse.bass as bass
    import concourse.tile as tile
    from concourse import mybir as mb

    F32, F32R, BF16 = mb.dt.float32, mb.dt.float32r, mb.dt.bfloat16
    AX = mb.AxisListType.X
    EXP = mb.ActivationFunctionType.Exp
    CPY = mb.ActivationFunctionType.Copy
    IDN = mb.ActivationFunctionType.Identity

    nc = bass.Bass("TRN2", target_bir_lowering=False, debug=False, num_devices=1)

    dq = nc.dram_tensor("query", (SQ, D), F32, kind="ExternalInput").ap()
    dk = nc.dram_tensor("key", (SKV, D), F32, kind="ExternalInput").ap()
    dv = nc.dram_tensor("value", (SKV, D), F32, kind="ExternalInput").ap()
    dW = {t: nc.dram_tensor(f"W{t}", (D, H), F32, kind="ExternalInput").ap()
          for t in "qkv"}
    db = {t: nc.dram_tensor(f"b{t}", (H,), F32, kind="ExternalInput").ap()
          for t in "qkv"}
    dident = nc.dram_tensor("ident", (P, P), F32, kind="ExternalInput").ap()
    dctx = nc.dram_tensor("context", (SQ, H), F32, kind="ExternalOutput").ap()
    dwei = nc.dram_tensor("weights", (SQ, SKV), F32, kind="ExternalOutput").ap()

    with tile.TileContext(nc) as tc:
        with tc.tile_pool(name="const", bufs=1) as const, \
             tc.tile_pool(name="big", bufs=1) as big, \
             tc.tile_pool(name="smal", bufs=2) as smal:

            ident_r = const.tile([P, P], F32R)
            nc.sync.dma_start(out=ident_r, in_=dident.bitcast(F32R))
            # bv broadcast to all partitions (step-0 partition DMA)
            bvb = const.tile([P, H], F32)
            bv_bcast = bass.AP(tensor=db["v"].tensor, offset=db["v"].offset,
                               ap=[[0, P]] + db["v"].ap)
            nc.sync.dma_start(out=bvb, in_=bv_bcast)

            # persistent activations
            qT = [big.tile([P, SQ], F32R, tag=f"qt{h}", name=f"qt{h}") for h in range(HC)]
            kT = [big.tile([P, SKV], F32R, tag=f"kt{h}", name=f"kt{h}") for h in range(HC)]
            v_n = big.tile([P, ST, 512], F32R, tag="vn")

            # ---------------- phase 0/1: weights, biases, inputs ----------
            with tc.tile_pool(name="wpool", bufs=1) as wpool, \
                 tc.tile_pool(name="xstage", bufs=4) as xstage, \
                 tc.tile_pool(name="xt", bufs=1) as xtp, \
                 tc.tile_pool(name="ps1", bufs=4, space="PSUM") as ps1, \
                 tc.tile_pool(name="ps1t", bufs=2, space="PSUM") as ps1t:


                xt3 = xtp.tile([P, DC, SQ], F32R, tag="xt3", name="xt3")
                xt = [xt3[:, c, :] for c in range(DC)]

                def load_transpose(dram_t):
                    # 4 transposed 128x128 blocks land side by side in one
                    # psum bank; one wide strided DVE copy moves all 4 to xt3.
                    xt_v = xt3.rearrange("p c (i q) -> p c i q", q=P)
                    for i in range(ST):
                        xs = xstage.tile([P, D], F32R, tag="xs")
                        nc.sync.dma_start(
                            out=xs, in_=dram_t[i * P:(i + 1) * P, :].bitcast(F32R))
                        pt = ps1t.tile([P, DC, P], F32R, tag="tp")
                        for c in range(DC):
                            nc.tensor.transpose(pt[:, c, :], xs[:, c * P:(c + 1) * P],
                                                ident_r)
                        nc.vector.tensor_copy(xt_v[:, :, i, :], pt)

                def project_T(t, dst):        # dst[h][:, s] = (x@Wt+bt).T
                    for h in range(HC):
                        pp = [ps1.tile([P, 512], F32, tag="proj", name=f"pp{cc}")
                              for cc in range(DC)]
                        for c in range(DC):
                            for cc in range(DC):
                                nc.tensor.matmul(
                                    pp[cc], w[t][:, c, h * P:(h + 1) * P],
                                    xt[c][:, cc * 512:(cc + 1) * 512],
                                    start=(c == 0), stop=(c == DC - 1))
                        for cc in range(DC):
                            nc.scalar.activation(
                                dst[h][:, cc * 512:(cc + 1) * 512], pp[cc], IDN,
                                bias=bT[t][:, h:h + 1], scale=1.0)

                w = {}
                for t in "qkv":
                    w[t] = wpool.tile([P, DC, H], F32R, tag=f"w{t}", name=f"w{t}")
                    nc.sync.dma_start(
                        out=w[t],
                        in_=dW[t].rearrange("(c p) h -> p c h", p=P).bitcast(F32R))
                bT = {}
                for t in "qk":
                    bT[t] = const.tile([P, HC], F32, name=f"bT{t}")
                    nc.sync.dma_start(
                        out=bT[t], in_=db[t].rearrange("(h p) -> p h", p=P))

                load_transpose(dk)
                project_T("k", kT)
                load_transpose(dq)
                project_T("q", qT)
                load_transpose(dv)
                for j in range(ST):           # v natural [kv, H] in bf16
                    pp = ps1.tile([P, 512], F32, tag="proj")
                    for c in range(DC):
                        nc.tensor.matmul(pp, xt[c][:, j * P:(j + 1) * P],
                                         w["v"][:, c, :],
                                         start=(c == 0), stop=(c == DC - 1))
                    nc.vector.tensor_tensor(out=v_n[:, j, :], in0=pp, in1=bvb,
                                            op=mb.AluOpType.add)

            # ---------------- phase 2: attention, software-pipelined -------
            # Emission order interleaves tile i+1's score matmuls with tile
            # i's exp/transpose/context tail so the PE never drains while ACT
            # computes exp.  PSUM: scores bufs=5 + transpose 2 + context 1 = 8.
            with tc.tile_pool(name="upool", bufs=2) as upool, \
                 tc.tile_pool(name="wout", bufs=2) as wout, \
                 tc.tile_pool(name="wtbs", bufs=8) as wtbs, \
                 tc.tile_pool(name="pssc", bufs=5, space="PSUM") as pssc, \
                 tc.tile_pool(name="pstb", bufs=2, space="PSUM") as pstb, \
                 tc.tile_pool(name="psctx", bufs=1, space="PSUM") as psctx:

                def emit_scores(i):
                    qs = slice(i * P, (i + 1) * P)
                    sc = [pssc.tile([P, 512], F32, tag="sc", name=f"sc{cc}")
                          for cc in range(NB)]
                    nmax4 = smal.tile([P, NB], F32, tag="nmax4")
                    for h in range(HC):
                        for cc in range(NB):
                            nc.tensor.matmul(sc[cc], qT[h][:, qs],
                                             kT[h][:, cc * 512:(cc + 1) * 512],
                                             start=(h == 0), stop=(h == HC - 1))
                    for cc in range(NB):
                        nc.vector.reduce_max(nmax4[:, cc:cc + 1], sc[cc], axis=AX)
                    return sc, nmax4

                def emit_tail(i, sc, nmax4):
                    qs = slice(i * P, (i + 1) * P)
                    nm = smal.tile([P, 1], F32, tag="nm")
                    nc.vector.reduce_max(nm, nmax4, axis=AX, negate=True)

                    U = upool.tile([P, SKV], F32R, tag="U")
                    sums4 = smal.tile([P, NB], F32, tag="sums4")
                    for cc in range(NB):
                        nc.scalar.activation(U[:, cc * 512:(cc + 1) * 512], sc[cc],
                                             EXP, bias=nm, scale=1.0,
                                             accum_out=sums4[:, cc:cc + 1])
                    ssum = smal.tile([P, 1], F32, tag="ssum")
                    nc.vector.reduce_sum(ssum, sums4, axis=AX)
                    rc = smal.tile([P, 1], F32, tag="rc")
                    nc.vector.reciprocal(rc, ssum)

                    # W fp32 out: normalize on ACT during SBUF->SBUF copy
                    Wt_ = wout.tile([P, SKV], F32, tag="W")
                    nc.scalar.activation(Wt_, U.bitcast(F32), CPY, bias=0.0,
                                         scale=rc)
                    nc.sync.dma_start(out=dwei[qs, :], in_=Wt_)

                    # PE-transpose unnormalized U (f32r), accumulate C' = U @ v
                    pc = psctx.tile([P, 512], F32, tag="ctx")
                    for g in range(4):
                        pt = pstb.tile([P, 512], F32R, tag="wtb")
                        for jj in range(4):
                            j = g * 4 + jj
                            nc.tensor.transpose(pt[:, jj * P:(jj + 1) * P],
                                                U[:, j * P:(j + 1) * P], ident_r)
                        wt_s = wtbs.tile([P, 512], F32R, tag="wtbs")
                        nc.vector.tensor_copy(wt_s, pt)
                        for jj in range(4):
                            j = g * 4 + jj
                            nc.tensor.matmul(pc, wt_s[:, jj * P:(jj + 1) * P],
                                             v_n[:, j, :],
                                             start=(j == 0), stop=(j == ST - 1))
                    # C = C' * recip, normalized during PSUM->SBUF copy on ACT
                    Ct = smal.tile([P, 512], F32, tag="C")
                    nc.scalar.activation(Ct, pc, CPY, bias=0.0, scale=rc)
                    nc.sync.dma_start(out=dctx[qs, :], in_=Ct)

                pending = None
                for i in range(ST):
                    sc_nm = emit_scores(i)
                    if pending is not None:
                        emit_tail(*pending)
                    pending = (i,) + sc_nm
                emit_tail(*pending)

    _patch_multiwaits(nc, mb)
    return nc


def kernel(**inputs):
    from concourse.bass_utils import run_bass_kernel_spmd

    import os
    if os.environ.get("LDWOPT", "1") == "1":
        _enable_ldw_opt()
    if "nc" not in _CACHE:
        _CACHE["nc"] = _build()
    nc = _CACHE["nc"]

    query = np.asarray(inputs["query"], dtype=np.float32)
    key = np.asarray(inputs["key"], dtype=np.float32)
    value = np.asarray(inputs["value"], dtype=np.float32)
    consts = {
        "Wq": np.asarray(inputs["Wq"], np.float32),
        "Wk": np.asarray(inputs["Wk"], np.float32),
        "Wv": np.asarray(inputs["Wv"], np.float32),
        "bq": np.asarray(inputs["bq"], np.float32),
        "bk": np.asarray(inputs["bk"], np.float32),
        "bv": np.asarray(inputs["bv"], np.float32),
        "ident": np.eye(P, dtype=np.float32),
    }
    in_maps = [dict(consts, query=query[b], key=key[b], value=value[b])
               for b in range(B)]
    res = run_bass_kernel_spmd(nc, in_maps, core_ids=list(range(B)),
                               **_CACHE.get("run_kwargs", {}))
    _CACHE["last_results"] = res
    context = np.stack([res.results[b]["context"] for b in range(B)])
    weights = np.stack([res.results[b]["weights"] for b in range(B)])
    return (context, weights)


# revision 24
# speedup vs baseline: 1.2592x; 1.2592x over previous
"""Fused attention kernel for Trainium2 (Bass/Tile), 8-core SPMD.

Problem: nn_Attention — per batch element b:
    q = query[b] @ Wq + bq ; k = key[b] @ Wk + bk ; v = value[b] @ Wv + bv
    W = softmax(q @ k.T) ; C = W @ v ; returns (C, W)

Sharding: data-parallel over batch (B=8 == 8 NeuronCores), one batch element
per core, projection weights replicated, no collectives.

Per-core dataflow (measured ~256 us/core on TRN2):
  1. Inputs stream in 4-tile groups: DMA -> PE transpose (fp32r) -> xT chunks.
     value is projected to v_n [Skv, H] (bf16); key/query to kT/qT [H, S]
     (fp32r, full fp32 data at 1 cycle/row on the PE).
  2. Per 128-row q-tile, software-pipelined with the next tile's score
     matmuls: S = qT.T @ kT into 4 PSUM banks (fp32r), row-max on DVE,
     exp(S - max) on ACT with per-bank row-sum accumulation -> U (bf16),
     W = U * 1/sum on ACT -> DMA out (fp32); U is PE-transposed (bf16) and
     C' = U.T-blocks @ v_n accumulates on PE; C = C' * 1/sum on ACT -> DMA.
  3. Query projection groups are interleaved with the first attention tiles
     so score matmuls start as soon as the first query chunk lands.

PSUM budget: score banks (5) + transpose staging (2) + context (1) = 8.
A post-pass splits multi-semaphore waits onto NoOps (this walrus build
accepts a single sync-wait per instruction).
"""

import numpy as np

B, SQ, SKV, D, H = 8, 2048, 2048, 512, 512
P = 128                 # partitions
ST = SQ // P            # 16 s-tiles
DC = D // P             # 4 contraction chunks
HC = H // P             # 4 h tiles
NB = SKV // 512         # 4 psum banks per score row

_CACHE = {}


def _patch_multiwaits(nc, mb):
    """walrus in this container rejects >1 sync-wait per instruction
    (setupSyncWait: "Too many sync wait commands").  Split extra waits onto
    preceding same-engine NoOps — engine streams are in-order so semantics
    are preserved."""
    for blk in nc.m.functions[0].blocks:
        insts = list(blk.instructions)
        new_insts, changed = [], False
        for inst in insts:
            si = getattr(inst, "sync_info", None)
            if si is not None and si.on_wait and len(si.on_wait) > 1:
                waits = list(si.on_wait)
                extra, keep = waits[:-1], waits[-1:]
                for k, w in enumerate(extra):
                    new_insts.append(mb.InstNoOp(
                        name=f"{inst.name}-ws{k}",
                        sync_info=mb.SyncInfo(on_wait=[w], on_update=[]),
                        bass_nofuse=True, engine=inst.engine))
                si.on_wait = keep
                changed = True
            new_insts.append(inst)
        if changed:
            blk.instructions = new_insts


def _enable_ldw_opt():
    """Flip walrus --enable-ldw-opt to true (elides redundant LDWEIGHTS)."""
    from concourse import bass_utils as bu
    if getattr(bu, "_ldw_patched", False):
        return
    orig = bu.run_command

    def patched(cmd, **kw):
        cmd = [c.replace("--enable-ldw-opt=false", "--enable-ldw-opt=true")
               if isinstance(c, str) else c for c in cmd]
        return orig(cmd, **kw)

    bu.run_command = patched
    bu._ldw_patched = True


def _build():
    import concourse.bass as bass
    import concourse.tile as tile
    from concourse import mybir as mb

    F32, F32R, BF16 = mb.dt.float32, mb.dt.float32r, mb.dt.bfloat16
    AX = mb.AxisListType.X
    EXP = mb.ActivationFunctionType.Exp
    CPY = mb.ActivationFunctionType.Copy
    IDN = mb.ActivationFunctionType.Identity

    nc = bass.Bass("TRN2", target_bir_lowering=False, debug=False, num_devices=1)

    dq = nc.dram_tensor("query", (SQ, D), F32, kind="ExternalInput").ap()
    dk = nc.dram_tensor("key", (SKV, D), F32, kind="ExternalInput").ap()
    dv = nc.dram_tensor("value", (SKV, D), F32, kind="ExternalInput").ap()
    dW = {t: nc.dram_tensor(f"W{t}", (D, H), F32, kind="ExternalInput").ap()
          for t in "qkv"}
    db = {t: nc.dram_tensor(f"b{t}", (H,), F32, kind="ExternalInput").ap()
          for t in "qkv"}
    dident = nc.dram_tensor("ident", (P, P), F32, kind="ExternalInput").ap()
    dctx = nc.dram_tensor("context", (SQ, H), F32, kind="ExternalOutput").ap()
    dwei = nc.dram_tensor("weights", (SQ, SKV), F32, kind="ExternalOutput").ap()

    NG = ST // 4  # 4 s-tile groups of 4 tiles (512 rows) per tensor

    with tile.TileContext(nc) as tc:
        with tc.tile_pool(name="const", bufs=1) as const, \
             tc.tile_pool(name="big", bufs=1) as big, \
             tc.tile_pool(name="wpool", bufs=2) as wpool, \
             tc.tile_pool(name="xstage", bufs=4) as xstage, \
             tc.tile_pool(name="xtc", bufs=2) as xtcp, \
             tc.tile_pool(name="upool", bufs=2) as upool, \
             tc.tile_pool(name="wout", bufs=2) as wout, \
             tc.tile_pool(name="wtbs", bufs=6) as wtbs, \
             tc.tile_pool(name="smal", bufs=2) as smal, \
             tc.tile_pool(name="psA", bufs=1, space="PSUM") as psA:

            ident_r = const.tile([P, P], F32R)
            nc.sync.dma_start(out=ident_r, in_=dident.bitcast(F32R))
            ident_b = const.tile([P, P], BF16)
            nc.vector.tensor_copy(ident_b, ident_r.bitcast(F32))
            bvb = const.tile([P, H], F32)
            bT = {}
            for t in "qk":
                bT[t] = const.tile([P, HC], F32, name=f"bT{t}")

            # PE warm-up: ~90 identity transposes (~7 us of dense PE work)
            # so the HAM clock-gate opens before real work arrives
            wupt = psA.tile([P, P], F32R, tag="tp", bufs=2, name="wupt")
            for _ in range(90):
                nc.tensor.transpose(wupt, ident_r, ident_r)
            wudst = smal.tile([P, P], F32R, tag="wu", name="wudst")
            nc.vector.tensor_copy(wudst, wupt)

            # persistent activations
            qT = [big.tile([P, SQ], F32R, tag=f"qt{h}", name=f"qt{h}")
                  for h in range(HC)]
            kT = [big.tile([P, SKV], F32R, tag=f"kt{h}", name=f"kt{h}")
                  for h in range(HC)]
            v_n = big.tile([P, ST, 512], BF16, tag="vn")

            # ------------- attention tile emitters (software-pipelined) ----
            def emit_scores(i):
                qs = slice(i * P, (i + 1) * P)
                sc = [psA.tile([P, 512], F32, tag="sc", bufs=5, name=f"sc{cc}")
                      for cc in range(NB)]
                nmax4 = smal.tile([P, NB], F32, tag="nmax4")
                for h in range(HC):
                    for cc in range(NB):
                        nc.tensor.matmul(sc[cc], qT[h][:, qs],
                                         kT[h][:, cc * 512:(cc + 1) * 512],
                                         start=(h == 0), stop=(h == HC - 1))
                for cc in range(NB):
                    nc.vector.reduce_max(nmax4[:, cc:cc + 1], sc[cc], axis=AX)
                return sc, nmax4

            def emit_tail(i, sc, nmax4):
                qs = slice(i * P, (i + 1) * P)
                nm = smal.tile([P, 1], F32, tag="nm")
                nc.vector.reduce_max(nm, nmax4, axis=AX, negate=True)

                U = upool.tile([P, SKV], BF16, tag="U")
                sums4 = smal.tile([P, NB], F32, tag="sums4")
                for cc in range(NB):
                    nc.scalar.activation(U[:, cc * 512:(cc + 1) * 512], sc[cc],
                                         EXP, bias=nm, scale=1.0,
                                         accum_out=sums4[:, cc:cc + 1])
                ssum = smal.tile([P, 1], F32, tag="ssum")
                nc.vector.reduce_sum(ssum, sums4, axis=AX)
                rc = smal.tile([P, 1], F32, tag="rc")
                nc.vector.reciprocal(rc, ssum)

                Wt_ = wout.tile([P, SKV], F32, tag="W")
                nc.scalar.activation(Wt_, U, CPY, bias=0.0,
                                     scale=rc)
                nc.sync.dma_start(out=dwei[qs, :], in_=Wt_)

                pc = psA.tile([P, 512], F32, tag="ctx", bufs=1)
                for g in range(4):
                    pt = psA.tile([P, 512], BF16, tag="tp", bufs=2)
                    for jj in range(4):
                        j = g * 4 + jj
                        nc.tensor.transpose(pt[:, jj * P:(jj + 1) * P],
                                            U[:, j * P:(j + 1) * P], ident_b)
                    wt_s = wtbs.tile([P, 512], BF16, tag="wtbs")
                    nc.vector.tensor_copy(wt_s, pt)
                    for jj in range(4):
                        j = g * 4 + jj
                        nc.tensor.matmul(pc, wt_s[:, jj * P:(jj + 1) * P],
                                         v_n[:, j, :],
                                         start=(j == 0), stop=(j == ST - 1))
                Ct = smal.tile([P, 512], F32, tag="C")
                nc.scalar.activation(Ct, pc, CPY, bias=0.0, scale=rc)
                nc.sync.dma_start(out=dctx[qs, :], in_=Ct)

            # ------------- phase 1: rolling 4-tile groups per tensor -------
            pending = [None]  # attention pipeline state

            def load_group(dram_t, g):
                """DMA 4 s-tiles, PE-transpose into a rolling xtc chunk."""
                xs4 = []
                for j4 in range(4):
                    xs = xstage.tile([P, D], F32R, tag="xs")
                    nc.sync.dma_start(
                        out=xs,
                        in_=dram_t[(4 * g + j4) * P:(4 * g + j4 + 1) * P, :]
                        .bitcast(F32R))
                    xs4.append(xs)
                return xs4

            def transpose_group(xs4, xtc):
                xv = xtc.rearrange("p c (j q) -> p c j q", q=P)
                for j4 in range(4):
                    pt = psA.tile([P, DC, P], F32R, tag="tp", bufs=2)
                    for c in range(DC):
                        nc.tensor.transpose(pt[:, c, :],
                                            xs4[j4][:, c * P:(c + 1) * P],
                                            ident_r)
                    nc.vector.tensor_copy(xv[:, :, j4, :], pt)

            def proj_kq(t, dst, wt, g):
                for h in range(HC):
                    pp = psA.tile([P, 512], F32, tag="sc", bufs=5)
                    for c in range(DC):
                        nc.tensor.matmul(pp, wt[:, c, h * P:(h + 1) * P],
                                         xtc[:, c, :],
                                         start=(c == 0), stop=(c == DC - 1))
                    nc.scalar.activation(dst[h][:, g * 512:(g + 1) * 512], pp,
                                         IDN, bias=bT[t][:, h:h + 1], scale=1.0)

            w = {}
            for gi, t in enumerate("vkq"):
                for g in range(NG):
                    xs4 = load_group({"v": dv, "k": dk, "q": dq}[t], g)
                    if g == 0:
                        # weight (and per-tensor aux) DMAs queue on the rings
                        # behind this tensor's first tile group
                        w[t] = wpool.tile([P, DC, H], F32R, tag="w",
                                          name=f"w{t}")
                        nc.sync.dma_start(
                            out=w[t],
                            in_=dW[t].rearrange("(c p) h -> p c h", p=P)
                            .bitcast(F32R))
                        if t == "v":
                            bv_bcast = bass.AP(
                                tensor=db["v"].tensor, offset=db["v"].offset,
                                ap=[[0, P]] + db["v"].ap)
                            nc.sync.dma_start(out=bvb, in_=bv_bcast)
                        else:
                            nc.sync.dma_start(
                                out=bT[t],
                                in_=db[t].rearrange("(h p) -> p h", p=P))
                    xtc = xtcp.tile([P, DC, 512], F32R, tag="xtc")
                    transpose_group(xs4, xtc)
                    if t == "v":
                        for j4 in range(4):
                            pp = psA.tile([P, 512], F32, tag="sc", bufs=5)
                            for c in range(DC):
                                nc.tensor.matmul(
                                    pp, xtc[:, c, j4 * P:(j4 + 1) * P],
                                    w[t][:, c, :],
                                    start=(c == 0), stop=(c == DC - 1))
                            nc.vector.tensor_tensor(
                                out=v_n[:, 4 * g + j4, :], in0=pp, in1=bvb,
                                op=mb.AluOpType.add)
                    elif t == "k":
                        proj_kq(t, kT, w[t], g)
                    else:
                        proj_kq(t, qT, w[t], g)
                        # attention tiles for this query chunk
                        for i in range(4 * g, 4 * g + 4):
                            sc_nm = emit_scores(i)
                            if pending[0] is not None:
                                emit_tail(*pending[0])
                            pending[0] = (i,) + sc_nm
            emit_tail(*pending[0])

    _patch_multiwaits(nc, mb)
    return nc


def kernel(**inputs):
    from concourse.bass_utils import run_bass_kernel_spmd

    import os
    if os.environ.get("LDWOPT", "0") == "1":
        _enable_ldw_opt()
    if "nc" not in _CACHE:
        _CACHE["nc"] = _build()
    nc = _CACHE["nc"]

    query = np.asarray(inputs["query"], dtype=np.float32)
    key = np.asarray(inputs["key"], dtype=np.float32)
    value = np.asarray(inputs["value"], dtype=np.float32)
    consts = {
        "Wq": np.asarray(inputs["Wq"], np.float32),
        "Wk": np.asarray(inputs["Wk"], np.float32),
        "Wv": np.asarray(inputs["Wv"], np.float32),
        "bq": np.asarray(inputs["bq"], np.float32),
        "bk": np.asarray(inputs["bk"], np.float32),
        "bv": np.asarray(inputs["bv"], np.float32),
        "ident": np.eye(P, dtype=np.float32),
    }
    in_maps = [dict(consts, query=query[b], key=key[b], value=value[b])
               for b in range(B)]
    res = run_bass_kernel_spmd(nc, in_maps, core_ids=list(range(B)),
                               **_CACHE.get("run_kwargs", {}))
    _CACHE["last_results"] = res
    context = np.stack([res.results[b]["context"] for b in range(B)])
    weights = np.stack([res.results[b]["weights"] for b in range(B)])
    return (context, weights)


# revision 25
# speedup vs baseline: 1.2853x; 1.0207x over previous
"""Fused attention kernel for Trainium2 (Bass/Tile), 8-core SPMD.

Problem: nn_Attention — per batch element b:
    q = query[b] @ Wq + bq ; k = key[b] @ Wk + bk ; v = value[b] @ Wv + bv
    W = softmax(q @ k.T) ; C = W @ v ; returns (C, W)

Sharding: data-parallel over batch (B=8 == 8 NeuronCores), one batch element
per core, projection weights replicated, no collectives.

Per-core dataflow (measured ~256 us/core on TRN2):
  1. Inputs stream in 4-tile groups: DMA -> PE transpose (fp32r) -> xT chunks.
     value is projected to v_n [Skv, H] (bf16); key/query to kT/qT [H, S]
     (fp32r, full fp32 data at 1 cycle/row on the PE).
  2. Per 128-row q-tile, software-pipelined with the next tile's score
     matmuls: S = qT.T @ kT into 4 PSUM banks (fp32r), row-max on DVE,
     exp(S - max) on ACT with per-bank row-sum accumulation -> U (bf16),
     W = U * 1/sum on ACT -> DMA out (fp32); U is PE-transposed (bf16) and
     C' = U.T-blocks @ v_n accumulates on PE; C = C' * 1/sum on ACT -> DMA.
  3. Query projection groups are interleaved with the first attention tiles
     so score matmuls start as soon as the first query chunk lands.

PSUM budget: score banks (5) + transpose staging (2) + context (1) = 8.
A post-pass splits multi-semaphore waits onto NoOps (this walrus build
accepts a single sync-wait per instruction).
"""

import numpy as np

B, SQ, SKV, D, H = 8, 2048, 2048, 512, 512
P = 128                 # partitions
ST = SQ // P            # 16 s-tiles
DC = D // P             # 4 contraction chunks
HC = H // P             # 4 h tiles
NB = SKV // 512         # 4 psum banks per score row

_CACHE = {}


def _patch_multiwaits(nc, mb):
    """walrus in this container rejects >1 sync-wait per instruction
    (setupSyncWait: "Too many sync wait commands").  Split extra waits onto
    preceding same-engine NoOps — engine streams are in-order so semantics
    are preserved."""
    for blk in nc.m.functions[0].blocks:
        insts = list(blk.instructions)
        new_insts, changed = [], False
        for inst in insts:
            si = getattr(inst, "sync_info", None)
            if si is not None and si.on_wait and len(si.on_wait) > 1:
                waits = list(si.on_wait)
                extra, keep = waits[:-1], waits[-1:]
                for k, w in enumerate(extra):
                    new_insts.append(mb.InstNoOp(
                        name=f"{inst.name}-ws{k}",
                        sync_info=mb.SyncInfo(on_wait=[w], on_update=[]),
                        bass_nofuse=True, engine=inst.engine))
                si.on_wait = keep
                changed = True
            new_insts.append(inst)
        if changed:
            blk.instructions = new_insts


def _enable_ldw_opt():
    """Flip walrus --enable-ldw-opt to true (elides redundant LDWEIGHTS)."""
    from concourse import bass_utils as bu
    if getattr(bu, "_ldw_patched", False):
        return
    orig = bu.run_command

    def patched(cmd, **kw):
        cmd = [c.replace("--enable-ldw-opt=false", "--enable-ldw-opt=true")
               if isinstance(c, str) else c for c in cmd]
        return orig(cmd, **kw)

    bu.run_command = patched
    bu._ldw_patched = True


def _build():
    import concourse.bass as bass
    import concourse.tile as tile
    from concourse import mybir as mb

    F32, F32R, BF16 = mb.dt.float32, mb.dt.float32r, mb.dt.bfloat16
    AX = mb.AxisListType.X
    EXP = mb.ActivationFunctionType.Exp
    CPY = mb.ActivationFunctionType.Copy
    IDN = mb.ActivationFunctionType.Identity

    nc = bass.Bass("TRN2", target_bir_lowering=False, debug=False, num_devices=1)

    dq = nc.dram_tensor("query", (SQ, D), F32, kind="ExternalInput").ap()
    dk = nc.dram_tensor("key", (SKV, D), F32, kind="ExternalInput").ap()
    dv = nc.dram_tensor("value", (SKV, D), F32, kind="ExternalInput").ap()
    dW = {t: nc.dram_tensor(f"W{t}", (D, H), F32, kind="ExternalInput").ap()
          for t in "qkv"}
    db = {t: nc.dram_tensor(f"b{t}", (H,), F32, kind="ExternalInput").ap()
          for t in "qkv"}
    dident = nc.dram_tensor("ident", (P, P), F32, kind="ExternalInput").ap()
    dctx = nc.dram_tensor("context", (SQ, H), F32, kind="ExternalOutput").ap()
    dwei = nc.dram_tensor("weights", (SQ, SKV), F32, kind="ExternalOutput").ap()

    NG = ST // 4  # 4 s-tile groups of 4 tiles (512 rows) per tensor

    with tile.TileContext(nc) as tc:
        with tc.tile_pool(name="const", bufs=1) as const, \
             tc.tile_pool(name="big", bufs=1) as big, \
             tc.tile_pool(name="wpool", bufs=2) as wpool, \
             tc.tile_pool(name="xstage", bufs=6) as xstage, \
             tc.tile_pool(name="xtc", bufs=2) as xtcp, \
             tc.tile_pool(name="upool", bufs=3) as upool, \
             tc.tile_pool(name="wout", bufs=2) as wout, \
             tc.tile_pool(name="wtbs", bufs=8) as wtbs, \
             tc.tile_pool(name="smal", bufs=3) as smal, \
             tc.tile_pool(name="psA", bufs=1, space="PSUM") as psA:

            ident_r = const.tile([P, P], F32R)
            nc.sync.dma_start(out=ident_r, in_=dident.bitcast(F32R))
            ident_b = const.tile([P, P], BF16)
            nc.vector.tensor_copy(ident_b, ident_r.bitcast(F32))
            bvb = const.tile([P, H], F32)
            bT = {}
            for t in "qk":
                bT[t] = const.tile([P, HC], F32, name=f"bT{t}")

            # PE warm-up: ~90 identity transposes (~7 us of dense PE work)
            # so the HAM clock-gate opens before real work arrives
            wupt = psA.tile([P, P], F32R, tag="tp", bufs=2, name="wupt")
            for _ in range(90):
                nc.tensor.transpose(wupt, ident_r, ident_r)
            wudst = smal.tile([P, P], F32R, tag="wu", name="wudst")
            nc.vector.tensor_copy(wudst, wupt)

            # persistent activations
            qT = [big.tile([P, SQ], F32R, tag=f"qt{h}", name=f"qt{h}")
                  for h in range(HC)]
            kT = [big.tile([P, SKV], F32R, tag=f"kt{h}", name=f"kt{h}")
                  for h in range(HC)]
            v_n = big.tile([P, ST, 512], BF16, tag="vn")

            # ------------- attention tile emitters (software-pipelined) ----
            def emit_scores(i):
                qs = slice(i * P, (i + 1) * P)
                sc = [psA.tile([P, 512], F32, tag="sc", bufs=5, name=f"sc{cc}")
                      for cc in range(NB)]
                nmax4 = smal.tile([P, NB], F32, tag="nmax4")
                for h in range(HC):
                    for cc in range(NB):
                        nc.tensor.matmul(sc[cc], qT[h][:, qs],
                                         kT[h][:, cc * 512:(cc + 1) * 512],
                                         start=(h == 0), stop=(h == HC - 1))
                for cc in range(NB):
                    nc.vector.reduce_max(nmax4[:, cc:cc + 1], sc[cc], axis=AX)
                return sc, nmax4

            def emit_tail(i, sc, nmax4):
                qs = slice(i * P, (i + 1) * P)
                nm = smal.tile([P, 1], F32, tag="nm")
                nc.vector.reduce_max(nm, nmax4, axis=AX, negate=True)

                U = upool.tile([P, SKV], BF16, tag="U")
                sums4 = smal.tile([P, NB], F32, tag="sums4")
                for cc in range(NB):
                    nc.scalar.activation(U[:, cc * 512:(cc + 1) * 512], sc[cc],
                                         EXP, bias=nm, scale=1.0,
                                         accum_out=sums4[:, cc:cc + 1])
                ssum = smal.tile([P, 1], F32, tag="ssum")
                nc.vector.reduce_sum(ssum, sums4, axis=AX)
                rc = smal.tile([P, 1], F32, tag="rc")
                nc.vector.reciprocal(rc, ssum)

                Wt_ = wout.tile([P, SKV], F32, tag="W")
                nc.scalar.activation(Wt_, U, CPY, bias=0.0,
                                     scale=rc)
                nc.sync.dma_start(out=dwei[qs, :], in_=Wt_)

                pc = psA.tile([P, 512], F32, tag="ctx", bufs=1)
                for g in range(4):
                    pt = psA.tile([P, 512], BF16, tag="tp", bufs=2)
                    for jj in range(4):
                        j = g * 4 + jj
                        nc.tensor.transpose(pt[:, jj * P:(jj + 1) * P],
                                            U[:, j * P:(j + 1) * P], ident_b)
                    wt_s = wtbs.tile([P, 512], BF16, tag="wtbs")
                    nc.vector.tensor_copy(wt_s, pt)
                    for jj in range(4):
                        j = g * 4 + jj
                        nc.tensor.matmul(pc, wt_s[:, jj * P:(jj + 1) * P],
                                         v_n[:, j, :],
                                         start=(j == 0), stop=(j == ST - 1))
                Ct = smal.tile([P, 512], F32, tag="C")
                nc.scalar.activation(Ct, pc, CPY, bias=0.0, scale=rc)
                nc.sync.dma_start(out=dctx[qs, :], in_=Ct)

            # ------------- phase 1: rolling 4-tile groups per tensor -------
            pending = [None]  # attention pipeline state

            def load_group(dram_t, g):
                """DMA 4 s-tiles, PE-transpose into a rolling xtc chunk."""
                xs4 = []
                for j4 in range(4):
                    xs = xstage.tile([P, D], F32R, tag="xs")
                    nc.sync.dma_start(
                        out=xs,
                        in_=dram_t[(4 * g + j4) * P:(4 * g + j4 + 1) * P, :]
                        .bitcast(F32R))
                    xs4.append(xs)
                return xs4

            def transpose_group(xs4, xtc):
                xv = xtc.rearrange("p c (j q) -> p c j q", q=P)
                for j4 in range(4):
                    pt = psA.tile([P, DC, P], F32R, tag="tp", bufs=2)
                    for c in range(DC):
                        nc.tensor.transpose(pt[:, c, :],
                                            xs4[j4][:, c * P:(c + 1) * P],
                                            ident_r)
                    nc.vector.tensor_copy(xv[:, :, j4, :], pt)

            def proj_kq(t, dst, wt, g):
                for h in range(HC):
                    pp = psA.tile([P, 512], F32, tag="sc", bufs=5)
                    for c in range(DC):
                        nc.tensor.matmul(pp, wt[:, c, h * P:(h + 1) * P],
                                         xtc[:, c, :],
                                         start=(c == 0), stop=(c == DC - 1))
                    nc.scalar.activation(dst[h][:, g * 512:(g + 1) * 512], pp,
                                         IDN, bias=bT[t][:, h:h + 1], scale=1.0)

            w = {}
            for gi, t in enumerate("vkq"):
                for g in range(NG):
                    xs4 = load_group({"v": dv, "k": dk, "q": dq}[t], g)
                    if g == 0:
                        # weight (and per-tensor aux) DMAs queue on the rings
                        # behind this tensor's first tile group
                        w[t] = wpool.tile([P, DC, H], F32R, tag="w",
                                          name=f"w{t}")
                        nc.sync.dma_start(
                            out=w[t],
                            in_=dW[t].rearrange("(c p) h -> p c h", p=P)
                            .bitcast(F32R))
                        if t == "v":
                            bv_bcast = bass.AP(
                                tensor=db["v"].tensor, offset=db["v"].offset,
                                ap=[[0, P]] + db["v"].ap)
                            nc.sync.dma_start(out=bvb, in_=bv_bcast)
                        else:
                            nc.sync.dma_start(
                                out=bT[t],
                                in_=db[t].rearrange("(h p) -> p h", p=P))
                    xtc = xtcp.tile([P, DC, 512], F32R, tag="xtc")
                    transpose_group(xs4, xtc)
                    if t == "v":
                        for j4 in range(4):
                            pp = psA.tile([P, 512], F32, tag="sc", bufs=5)
                            for c in range(DC):
                                nc.tensor.matmul(
                                    pp, xtc[:, c, j4 * P:(j4 + 1) * P],
                                    w[t][:, c, :],
                                    start=(c == 0), stop=(c == DC - 1))
                            nc.vector.tensor_tensor(
                                out=v_n[:, 4 * g + j4, :], in0=pp, in1=bvb,
                                op=mb.AluOpType.add)
                    elif t == "k":
                        proj_kq(t, kT, w[t], g)
                    else:
                        proj_kq(t, qT, w[t], g)
                        # attention tiles for this query chunk
                        for i in range(4 * g, 4 * g + 4):
                            sc_nm = emit_scores(i)
                            if pending[0] is not None:
                                emit_tail(*pending[0])
                            pending[0] = (i,) + sc_nm
            emit_tail(*pending[0])

    _patch_multiwaits(nc, mb)
    return nc


def kernel(**inputs):
    from concourse.bass_utils import run_bass_kernel_spmd

    import os
    if os.environ.get("LDWOPT", "0") == "1":
        _enable_ldw_opt()
    if "nc" not in _CACHE:
        _CACHE["nc"] = _build()
    nc = _CACHE["nc"]

    query = np.asarray(inputs["query"], dtype=np.float32)
    key = np.asarray(inputs["key"], dtype=np.float32)
    value = np.asarray(inputs["value"], dtype=np.float32)
    consts = {
        "Wq": np.asarray(inputs["Wq"], np.float32),
        "Wk": np.asarray(inputs["Wk"], np.float32),
        "Wv": np.asarray(inputs["Wv"], np.float32),
        "bq": np.asarray(inputs["bq"], np.float32),
        "bk": np.asarray(inputs["bk"], np.float32),
        "bv": np.asarray(inputs["bv"], np.float32),
        "ident": np.eye(P, dtype=np.float32),
    }
    in_maps = [dict(consts, query=query[b], key=key[b], value=value[b])
               for b in range(B)]
    res = run_bass_kernel_spmd(nc, in_maps, core_ids=list(range(B)),
                               **_CACHE.get("run_kwargs", {}))
    _CACHE["last_results"] = res
    context = np.stack([res.results[b]["context"] for b in range(B)])
    weights = np.stack([res.results[b]["weights"] for b in range(B)])
    return (context, weights)
